# revision 1
# baseline (speedup 1.0000x reference)
"""Trainium2 Bass kernel for nn_ActQuantWrapper (per-token 4-bit fake-quant + Linear).

Strategy (8 NeuronCores, SPMD, no collectives):
  - Shard x along the sequence axis: 1024 tokens per core; weight/bias replicated.
  - Host prep: W^T (+bias row appended) cast to bf16; q / not-fp index masks as uint8.
  - Per core:
      * masked x (q-features, else 0) via copy_predicated into a persistent xm tile
        whose non-q lanes provably return to 0 after each tile's quant chain
      * per-token stats: min/max reduces (0 always included via masked lanes,
        matching reference's min(.,0)/max(.,0))
      * fake-quant with per-partition scalars on DVE; RNE round via the
        +/-1.5*2^23 magic constant fused into dual-op tensor_scalar
      * merge: x16 = bf16(x); copy_predicated(x16, notfp, dq16) -> mixed (bf16)
      * DMA-xbar transpose (dma_start_transpose) into [feature, token] tiles
      * bf16 matmul: stationary = mixed^T tile (128x128), streaming = W^T chunk
        (N=512), PSUM accum over 32 feature tiles; bias via a K=1 ones-row matmul;
        ACT drains PSUM -> SBUF and issues the output DMAs.
  - Unequal token groups (128/384/512 tokens): tiny first group minimizes the
    PE idle ramp while later groups' quant hides under earlier groups' matmuls.
    W^T streams from HBM once per group.
  - DMA issue streams are split per engine so semaphore waits don't serialize
    unrelated transfers: x loads on GpSimd (SWDGE), W chunks + transposes on
    Sync (HWDGE), outputs on Scalar (HWDGE).
"""

import sys
import numpy as np
import ml_dtypes

sys.path.insert(0, "/opt/trn_rl_repo")

import concourse.bass as bass  # noqa: E402
import concourse.mybir as mybir  # noqa: E402
import concourse.tile as tile  # noqa: E402
from concourse import bacc  # noqa: E402

F32 = mybir.dt.float32
BF16 = mybir.dt.bfloat16
U8 = mybir.dt.uint8

N_CORES = 8
S_FULL, D, O = 8192, 4096, 4096
T = S_FULL // N_CORES          # tokens per core
MAGIC = 12582912.0             # 1.5 * 2**23 : RNE round-to-int for |v| < 2**22
MAXQ = 15.0
RANGE_FLOOR = 1e-30            # degenerate all-zero token guard (dq ends up 0 anyway)

N_TT = T // 128                # token tiles per core
GROUP_TTS = [2, 3, 3]          # token tiles per group (sums to N_TT)
CHUNK = 512                    # output-feature chunk per W^T stream tile
N_CH = O // CHUNK
N_DT = D // 128                # feature (contraction) tiles
MT_BUFS = 6                    # max live mixed^T tiles (g_i MM + g_{i+1} quant)

_CACHE = {}


def _build_bass(mode="full"):
    nc = bacc.Bacc("TRN2", target_bir_lowering=False, debug=False,
                   enable_asserts=True, num_devices=N_CORES)
    x_ap = nc.dram_tensor("x", [T, D], F32, kind="ExternalInput").ap()
    wt_ap = nc.dram_tensor("wt", [D + 1, O], BF16, kind="ExternalInput").ap()
    qm_ap = nc.dram_tensor("qmask", [1, D], U8, kind="ExternalInput").ap()
    fm_ap = nc.dram_tensor("notfp", [1, D], U8, kind="ExternalInput").ap()
    bf_ap = nc.dram_tensor("biasf", [1, O], F32, kind="ExternalInput").ap()
    out_ap = nc.dram_tensor("out", [T, O], F32, kind="ExternalOutput").ap()

    with tile.TileContext(nc) as tc:
        _kernel_body(tc, out_ap, x_ap, wt_ap, qm_ap, fm_ap, bf_ap, mode)
    nc.compile()
    return nc


def _kernel_body(tc, out_ap, x_ap, wt_ap, qm_ap, fm_ap, bf_ap, mode="full"):
    from contextlib import ExitStack
    nc = tc.nc
    A = mybir.AluOpType

    with ExitStack() as ctx:
        singles = ctx.enter_context(tc.tile_pool(name="singles", bufs=1))
        xp = ctx.enter_context(tc.tile_pool(name="xp", bufs=2))
        x16p = ctx.enter_context(tc.tile_pool(name="x16p", bufs=2))
        pp = ctx.enter_context(tc.tile_pool(name="pp", bufs=2))
        mtp = ctx.enter_context(tc.tile_pool(name="mtp", bufs=MT_BUFS))
        wcp = ctx.enter_context(tc.tile_pool(name="wcp", bufs=2))
        bbp = ctx.enter_context(tc.tile_pool(name="bbp", bufs=2))
        osp = ctx.enter_context(tc.tile_pool(name="osp", bufs=2))
        pmm = ctx.enter_context(tc.tile_pool(name="pmm", bufs=4, space="PSUM"))

        # --- constants (broadcast index masks across all 128 partitions) ---
        qmask_b = singles.tile([128, D], U8)
        nc.gpsimd.dma_start(out=qmask_b, in_=bass.AP(
            tensor=qm_ap.tensor, offset=qm_ap.offset, ap=[[0, 128], qm_ap.ap[1]]))
        notfp_b = singles.tile([128, D], U8)
        nc.gpsimd.dma_start(out=notfp_b, in_=bass.AP(
            tensor=fm_ap.tensor, offset=fm_ap.offset, ap=[[0, 128], fm_ap.ap[1]]))
        # persistent masked-x work tile: non-q lanes are 0 after every tile's
        # chain (0 -> *inv+MAGIC=MAGIC -> -MAGIC,min hi = 0), so one memset
        # suffices for the whole kernel.
        xm = singles.tile([128, D], F32)
        nc.vector.memset(xm, 0.0)
        dq16 = singles.tile([128, D], BF16)

        def load_wtc(ch):
            col = ch * CHUNK
            wtc = wcp.tile([128, N_DT, CHUNK], BF16, tag="wtc")
            nc.scalar.dma_start(
                out=wtc,
                in_=wt_ap[0:D, col:col + CHUNK].rearrange("(j p) c -> p j c", p=128))
            bias_b = bbp.tile([128, CHUNK], F32, tag="bb")
            nc.gpsimd.dma_start(out=bias_b, in_=bass.AP(
                tensor=bf_ap.tensor, offset=bf_ap.offset + col,
                ap=[[0, 128], [1, CHUNK]]))
            return wtc, bias_b

        row0 = 0
        for g, g_tts in enumerate(GROUP_TTS):
            wtcs = {}
            if mode == "full" and g == 0:
                # nothing else contends for the SP stream yet: prefetch first
                for ch in range(2):
                    wtcs[ch] = load_wtc(ch)
            mts = []
            for tt in range(g_tts):
                row = row0 + tt * 128
                xt = xp.tile([128, D], F32, tag="x")
                nc.gpsimd.dma_start(out=xt, in_=x_ap[row:row + 128, :])
                x16 = x16p.tile([128, D], BF16, tag="x16")
                nc.vector.tensor_copy(x16, xt)

                # xm = x where q-feature else 0 (masked copy into persistent tile)
                nc.vector.copy_predicated(xm, qmask_b, xt)

                # rmax >= 0 >= rmin guaranteed: masked lanes contribute 0
                rmax = pp.tile([128, 1], F32, tag="rmax")
                rmin = pp.tile([128, 1], F32, tag="rmin")
                nc.vector.tensor_reduce(rmax, xm, axis=mybir.AxisListType.X, op=A.max)
                nc.vector.tensor_reduce(rmin, xm, axis=mybir.AxisListType.X, op=A.min)

                # per-token quant params (tiny [128,1] columns)
                rng = pp.tile([128, 1], F32, tag="rng")
                nc.vector.tensor_tensor(rng, rmax, rmin, A.subtract)
                s = pp.tile([128, 1], F32, tag="s")       # scale = range/15
                nc.vector.tensor_scalar(s, rng, RANGE_FLOOR, 1.0 / MAXQ, A.max, A.mult)
                inv = pp.tile([128, 1], F32, tag="inv")
                nc.vector.reciprocal(inv, s)
                lop = pp.tile([128, 1], F32, tag="lop")   # lo = round(xmin/scale) = -zero
                nc.vector.tensor_scalar(lop, rmin, inv, MAGIC, A.mult, A.add)
                lo = pp.tile([128, 1], F32, tag="lo")
                nc.vector.tensor_scalar(lo, lop, MAGIC, None, A.subtract)
                hi = pp.tile([128, 1], F32, tag="hi")
                nc.vector.tensor_scalar(hi, lo, MAXQ, None, A.add)

                # quantize: xm <- xm/scale + MAGIC ; xm <- min(xm-MAGIC, hi) ;
                # dq16 <- bf16( max(xm, lo) * scale )
                nc.vector.tensor_scalar(xm, xm, inv, MAGIC, A.mult, A.add)
                nc.vector.tensor_scalar(xm, xm, MAGIC, hi, A.subtract, A.min)
                nc.vector.tensor_scalar(dq16, xm, lo, s, A.max, A.mult)
                # mixed16 = dq16 except fp features keep x: overwrite into x16
                nc.vector.copy_predicated(x16, notfp_b, dq16)

                if mode == "quant":
                    ofl = osp.tile([128, D], F32, tag="ofl")
                    nc.vector.tensor_copy(ofl, x16)
                    nc.scalar.dma_start(out=out_ap[row:row + 128, :], in_=ofl)
                    continue

                # DMA-xbar block-transpose: mt[p, j, t] = x16[t, 128*j + p]
                mt = mtp.tile([128, N_DT, 128], BF16, tag="mt")
                mts.append(mt)
                nc.sync.dma_start_transpose(mt, x16)

            if mode == "full" and g > 0:
                for ch in range(2):
                    wtcs[ch] = load_wtc(ch)
            if mode != "full":
                row0 += g_tts * 128
                continue

            # matmul phase: stream W^T chunks, accumulate over feature tiles
            for ch in range(N_CH):
                col = ch * CHUNK
                wtc, bias_b = wtcs.pop(ch)
                if ch + 2 < N_CH:
                    wtcs[ch + 2] = load_wtc(ch + 2)

                for tt in range(g_tts):
                    row = row0 + tt * 128
                    ps = pmm.tile([128, CHUNK], F32, tag="mm")
                    for j in range(N_DT):
                        nc.tensor.matmul(ps, lhsT=mts[tt][:, j, :], rhs=wtc[:, j, :],
                                         start=(j == 0), stop=(j == N_DT - 1))
                    ost = osp.tile([128, CHUNK], F32, tag="ost")
                    nc.vector.tensor_tensor(ost, ps, bias_b, A.add)
                    nc.scalar.dma_start(out=out_ap[row:row + 128, col:col + CHUNK],
                                        in_=ost)
            row0 += g_tts * 128


def _get_nc():
    if "nc" not in _CACHE:
        _CACHE["nc"] = _build_bass()
    return _CACHE["nc"]


def _prep_in_maps(x, weight, bias, q_idx, fp_idx):
    x = np.ascontiguousarray(np.asarray(x, dtype=np.float32)).reshape(S_FULL, D)
    weight = np.asarray(weight, dtype=np.float32)
    bias = np.asarray(bias, dtype=np.float32)
    q_idx = np.asarray(q_idx).astype(np.int64)
    fp_idx = np.asarray(fp_idx).astype(np.int64)

    wt = np.empty((D + 1, O), dtype=ml_dtypes.bfloat16)
    wt[:D] = weight.T.astype(ml_dtypes.bfloat16)
    wt[D] = bias.astype(ml_dtypes.bfloat16)

    qmask = np.zeros((1, D), dtype=np.uint8)
    qmask[0, q_idx] = 1
    notfp = np.ones((1, D), dtype=np.uint8)
    notfp[0, fp_idx] = 0

    shared = {"wt": wt, "qmask": qmask, "notfp": notfp,
              "biasf": np.ascontiguousarray(bias[None, :])}
    return [
        {"x": np.ascontiguousarray(x[c * T:(c + 1) * T]), **shared}
        for c in range(N_CORES)
    ]


def kernel(x, weight, bias, q_idx, fp_idx):
    from concourse import bass_utils
    bass_utils.upload_artifacts = lambda tmpdir: "local://none"

    nc = _get_nc()
    in_maps = _prep_in_maps(x, weight, bias, q_idx, fp_idx)
    res = bass_utils.run_bass_kernel_spmd(
        nc, in_maps, core_ids=list(range(N_CORES)))
    out = np.concatenate([res.results[c]["out"] for c in range(N_CORES)], axis=0)
    return out.reshape(1, S_FULL, O)

